# revision 1
# baseline (speedup 1.0000x reference)
"""Trainium2 Bass kernel for nn_Attention_19104014533260.

Dense transformer attention block:
  qkv 1x1 conv + BN -> 4-head attention over 4096 pixels (d_k=32, d_v=64)
  -> + depthwise 3x3 conv(v) + BN -> proj 1x1 conv + BN.

Sharding: queries (pixel dim, n=4096) split across 8 cores; each core computes
all heads/batches for its 512-pixel slice. k/v are computed from the full
(replicated) x on every core, so there are no collectives.

Layout choices per core:
  - S^T[j, i] tiles [128 j-part, 512 i-free] from matmul(lhsT=k[:, jchunk],
    rhs=q_masked) with K=128 (q is masked per head so the other heads' rows
    contribute 0).
  - exp on ScalarE (PSUM -> SBUF, bf16 out) -- this is the bottleneck engine.
  - y = v @ P^T via bf16 matmuls accumulating over j-chunks; an appended
    ones-row in vT makes PSUM row 64 the softmax denominator for free.
  - depthwise 3x3 pe conv as 9 fused (v*w + acc) VectorE ops on a zero-padded
    v patch; BN biases folded on the host everywhere.
"""

import sys

sys.path.insert(0, "/opt/trn_rl_repo")

import numpy as np

import concourse.bass as bass
import concourse.bacc as bacc_mod
import concourse.tile as tile
from concourse import mybir

F32R = mybir.dt.float32r
F32 = mybir.dt.float32
BF16 = mybir.dt.bfloat16
F16 = mybir.dt.float16

EPS = 1e-5
NCORES = 8
N = 4096           # pixels per batch
SLICE = N // NCORES  # 512 query columns per core
B = 2
HEADS = 4
KD = 32            # key dim
HD = 64            # head (value) dim
DIM = 256

_CACHE = {}


def _build_program(debug=False):
    """Build (once) the single SPMD Bass program run on every core."""
    nc = bacc_mod.Bacc()

    x_d = nc.dram_tensor("x", [B, 2, 128, N], F32R, kind="ExternalInput")
    xr_d = nc.dram_tensor("xr", [B, 2, 128, 640], F32R, kind="ExternalInput")
    wk_d = nc.dram_tensor("wk", [128, 2, 128], F32R, kind="ExternalInput")
    wq_d = nc.dram_tensor("wq", [128, 2, 128], F32R, kind="ExternalInput")
    wva_d = nc.dram_tensor("wva", [128, 2, 260], F32R, kind="ExternalInput")
    wv_d = nc.dram_tensor("wv", [128, 2, 2, 128], F32R, kind="ExternalInput")
    wp_d = nc.dram_tensor("wp", [128, 2, 2, 128], F32R, kind="ExternalInput")
    bk_d = nc.dram_tensor("bk", [128, 1], F32, kind="ExternalInput")
    bq_d = nc.dram_tensor("bq", [128, 1], F32, kind="ExternalInput")
    bva_d = nc.dram_tensor("bva", [1, 260], F32, kind="ExternalInput")
    bv_d = nc.dram_tensor("bv", [128, 2], F32, kind="ExternalInput")
    bp_d = nc.dram_tensor("bp", [128, 2], F32, kind="ExternalInput")
    w9_d = nc.dram_tensor("w9", [128, 2, 9], F32, kind="ExternalInput")
    hmask_d = nc.dram_tensor("hmask", [128, 4], F32, kind="ExternalInput")
    vmask_d = nc.dram_tensor("vmask", [1, 640], F32, kind="ExternalInput")
    out_d = nc.dram_tensor("out", [B, 2, 128, SLICE], F32, kind="ExternalOutput")
    if debug:
        dk_d = nc.dram_tensor("dk", [128, 8, 512], F16, kind="ExternalOutput")
        dq_d = nc.dram_tensor("dq", [128, 4, 512], F16, kind="ExternalOutput")
        dva_d = nc.dram_tensor("dva", [128, 32, 260], F16, kind="ExternalOutput")
        dvp_d = nc.dram_tensor("dvp", [2, 128, 660], F32, kind="ExternalOutput")
        dz_d = nc.dram_tensor("dz", [2, 128, 512], F32, kind="ExternalOutput")
        dy_d = nc.dram_tensor("dy", [2, 128, 512], F32, kind="ExternalOutput")

    def bcast_rows(dram_ap, rows):
        # DRAM row tensor -> partition-replicated AP
        return bass.AP(
            tensor=dram_ap.tensor,
            offset=dram_ap.offset,
            ap=[[0, rows]] + [list(p) for p in dram_ap.ap[1:]],
        )

    with tile.TileContext(nc) as tc:
        with (
            tc.tile_pool(name="singles", bufs=1) as singles,
            tc.tile_pool(name="xp", bufs=8) as xp,
            tc.tile_pool(name="xrp", bufs=2) as xrp,
            tc.tile_pool(name="kp", bufs=16) as kp,
            tc.tile_pool(name="qp", bufs=4) as qp,
            tc.tile_pool(name="vap", bufs=64) as vap,
            tc.tile_pool(name="vpp", bufs=3) as vpp,
            tc.tile_pool(name="ep", bufs=8) as ep,
            tc.tile_pool(name="zp", bufs=8) as zp,
            tc.tile_pool(name="rp", bufs=4) as rp,
            tc.tile_pool(name="rfp", bufs=4) as rfp,
            tc.tile_pool(name="ytp", bufs=4) as ytp,
            tc.tile_pool(name="op", bufs=4) as op_,
            tc.tile_pool(name="drp", bufs=4, space="DRAM") as drp,
            tc.tile_pool(name="spool", bufs=2, space="PSUM") as spool,
            tc.tile_pool(name="ypool", bufs=2, space="PSUM") as ypool,
            tc.tile_pool(name="gpool", bufs=2, space="PSUM") as gpool,
        ):
            wk_s = singles.tile([128, 2, 128], F32R, tag="wk")
            wq_s = singles.tile([128, 2, 128], F32R, tag="wq")
            wva_s = singles.tile([128, 2, 260], F32R, tag="wva")
            wv_s = singles.tile([128, 2, 2, 128], F32R, tag="wv")
            wp_s = singles.tile([128, 2, 2, 128], F32R, tag="wp")
            bk_s = singles.tile([128, 1], F32, tag="bk")
            bq_s = singles.tile([128, 1], F32, tag="bq")
            bva_s = singles.tile([128, 260], F32, tag="bva")
            bv_s = singles.tile([128, 2], F32, tag="bv")
            bp_s = singles.tile([128, 2], F32, tag="bp")
            w9_s = singles.tile([128, 2, 9], F32, tag="w9")
            hmask_s = singles.tile([128, 4], F32, tag="hmask")
            vmask_s = singles.tile([128, 640], F32, tag="vmask")
            ones_s = singles.tile([1, 128], F32, tag="ones")
            nc.vector.memset(ones_s, 1.0)
            def load_weights_crit():
                # q/k-path weights ride the sync queue ahead of the x tiles
                for t, d in [(wq_s, wq_d), (wk_s, wk_d), (bq_s, bq_d),
                             (hmask_s, hmask_d), (bk_s, bk_d)]:
                    nc.sync.dma_start(t, d.ap())

            def load_weights_bulk():
                for t, d in [(wva_s, wva_d), (wv_s, wv_d), (bv_s, bv_d),
                             (wp_s, wp_d), (bp_s, bp_d), (w9_s, w9_d)]:
                    nc.gpsimd.dma_start(t, d.ap())
                nc.gpsimd.dma_start(bva_s, bcast_rows(bva_d.ap(), 128))
                nc.gpsimd.dma_start(vmask_s, bcast_rows(vmask_d.ap(), 128))

            TAPS = [(dy, dx) for dy in (-1, 0, 1) for dx in (-1, 0, 1)]

            state = {}

            def load_x(b):
                xrt = xrp.tile([128, 2, 640], F32R, tag="xr", name=f"xr{b}")
                nc.gpsimd.dma_start(
                    xrt, xr_d.ap()[b].rearrange("a p c -> p a c"))
                xts = []
                for n8 in range(8):
                    xt = xp.tile([128, 2, 512], F32R, tag="x", name=f"x{b}_{n8}")
                    nc.sync.dma_start(
                        xt, x_d.ap()[b, :, :, n8 * 512:(n8 + 1) * 512]
                        .rearrange("a p c -> p a c"))
                    xts.append(xt)
                state[b] = dict(xrt=xrt, xts=xts)

            def qkv(b):
                xrt = state[b]["xrt"]
                xts = state[b]["xts"]

                qts = [qp.tile([128, 2, 512], F16, tag="q", name=f"q{b}_{p}")
                       for p in range(2)]
                psq = gpool.tile([128, 512], F32, tag="g", name=f"qps{b}")
                for kc in range(2):
                    nc.tensor.matmul(psq, wq_s[:, kc, :], xrt[:, kc, 64:576],
                                     start=(kc == 0), stop=(kc == 1))
                def qcopy(h):
                    nc.vector.tensor_scalar(
                        out=qts[h // 2][:, h % 2, :], in0=psq,
                        scalar1=bq_s, scalar2=hmask_s[:, h:h + 1],
                        op0=mybir.AluOpType.add, op1=mybir.AluOpType.mult)

                def kchunk(n8):
                    ktc = kp.tile([128, 512], F16, tag="k", name=f"k{b}_{n8}")
                    ps = gpool.tile([128, 512], F32, tag="g", name=f"kps{b}_{n8}")
                    for kc in range(2):
                        nc.tensor.matmul(ps, wk_s[:, kc, :], xts[n8][:, kc, :],
                                         start=(kc == 0), stop=(kc == 1))
                    nc.vector.tensor_scalar_add(out=ktc, in0=ps, scalar1=bk_s)
                    return ktc

                # pair-0 q copies and the first k chunk lead so the first
                # S-matmul isn't queued behind all four q mask-copies on DVE
                qcopy(0)
                qcopy(1)
                kts = [kchunk(0)]
                qcopy(2)
                qcopy(3)
                for n8 in range(1, 8):
                    kts.append(kchunk(n8))

                vats = []
                for n32 in range(32):
                    vac = vap.tile([128, 260], F16, tag="va", name=f"va{b}_{n32}")
                    ps = gpool.tile([128, 260], F32, tag="g", name=f"vaps{b}_{n32}")
                    n8, sub = divmod(n32, 4)
                    for kc in range(2):
                        nc.tensor.matmul(
                            ps, xts[n8][:, kc, sub * 128:(sub + 1) * 128],
                            wva_s[:, kc, :], start=(kc == 0), stop=(kc == 1))
                    nc.vector.tensor_tensor(out=vac, in0=ps,
                                            in1=bva_s[:, :260],
                                            op=mybir.AluOpType.add)
                    vats.append(vac)

                vps = []
                for mc in range(2):
                    vpt = vpp.tile([128, 660], F32, tag="vp", name=f"vp{b}_{mc}")
                    nc.vector.memset(vpt, 0.0)
                    vpv = vpt.rearrange("p (r c) -> p r c", c=66)
                    for (c0, w, r0, nr) in [(0, 512, 0, 8), (512, 128, 8, 2)]:
                        ps = gpool.tile([128, w], F32, tag="g",
                                        name=f"vps{b}_{mc}_{c0}")
                        for kc in range(2):
                            nc.tensor.matmul(ps, wv_s[:, kc, mc, :],
                                             xrt[:, kc, c0:c0 + w],
                                             start=(kc == 0), stop=(kc == 1))
                        nc.vector.scalar_tensor_tensor(
                            out=vpv[:, r0:r0 + nr, 1:65],
                            in0=ps.rearrange("p (r c) -> p r c", c=64),
                            scalar=bv_s[:, mc:mc + 1],
                            in1=vmask_s[:, c0:c0 + w]
                            .rearrange("p (r c) -> p r c", c=64),
                            op0=mybir.AluOpType.add, op1=mybir.AluOpType.mult)
                    vps.append(vpt)
                state[b].update(kts=kts, qts=qts, vats=vats, vps=vps, zts={},
                                zys={})
                if debug and b == 0:
                    for n8 in range(8):
                        nc.sync.dma_start(dk_d.ap()[:, n8], kts[n8])
                    nc.sync.dma_start(dq_d.ap()[:, 0:2], qts[0])
                    nc.sync.dma_start(dq_d.ap()[:, 2:4], qts[1])
                    for n32 in range(32):
                        nc.sync.dma_start(dva_d.ap()[:, n32], vats[n32])
                    for mc in range(2):
                        nc.sync.dma_start(dvp_d.ap()[mc], vps[mc])

            def attn(b, ph):
                st = state[b]
                kts, vats = st["kts"], st["vats"]
                qt = st["qts"][ph]
                yts = [ypool.tile([128, 512], F32, tag="y", name=f"y{b}_{ph}_{hh}")
                       for hh in range(2)]
                ets = {}
                for jc in range(33):
                    if jc < 32:
                        sp = spool.tile([128, 1024], F32, tag="s",
                                        name=f"s{b}_{ph}_{jc}")
                        j8, jsub = divmod(jc, 4)
                        for hh in range(2):
                            h = 2 * ph + hh
                            nc.tensor.matmul(
                                sp[:, hh * 512:(hh + 1) * 512],
                                kts[j8][:, jsub * 128:(jsub + 1) * 128],
                                qt[:, hh, :], start=True, stop=True)
                        et = ep.tile([128, 1024], F16, tag="e",
                                     name=f"e{b}_{ph}_{jc}")
                        nc.scalar.activation(
                            et, sp, mybir.ActivationFunctionType.Exp)
                        ets[jc] = et
                    if jc >= 1:
                        et = ets.pop(jc - 1)
                        for hh in range(2):
                            h = 2 * ph + hh
                            nc.tensor.matmul(
                                yts[hh][0:65, :],
                                vats[jc - 1][:, 65 * h:65 * h + 65],
                                et[:, hh * 512:(hh + 1) * 512],
                                start=(jc == 1), stop=(jc == 32))
                yss = []
                for hh in range(2):
                    ys = ytp.tile([65, 512], F32, tag="yt", name=f"ys{b}_{ph}_{hh}")
                    nc.vector.tensor_copy(out=ys, in_=yts[hh][0:65, :])
                    yss.append(ys)
                st[f"y{ph}"] = yss

            def conv(b, ph):
                st = state[b]
                zt = zp.tile([128, 512], F32R, tag="z", name=f"z{b}_{ph}")
                zv = zt.rearrange("p (r c) -> p r c", c=64)
                vpv = st["vps"][ph].rearrange("p (r c) -> p r c", c=66)
                for t, (dy, dx) in enumerate(TAPS):
                    nc.vector.scalar_tensor_tensor(
                        out=zv,
                        in0=vpv[:, 1 + dy:9 + dy, 1 + dx:65 + dx],
                        scalar=w9_s[:, ph, t:t + 1], in1=zv,
                        op0=mybir.AluOpType.mult,
                        op1=(mybir.AluOpType.bypass if t == 0
                             else mybir.AluOpType.add))
                st["zts"][ph] = zt

            def norm(b, ph):
                st = state[b]
                yss = st[f"y{ph}"]
                zy = zp.tile([128, 512], F32R, tag="z", name=f"zy{b}_{ph}")
                for hh in range(2):
                    rt0 = rp.tile([1, 512], F32, tag="r0", name=f"r0_{b}{ph}{hh}")
                    # reciprocal_approx_fast misreads nonzero-base inputs;
                    # copy the sum row down to partition 0 first.
                    nc.vector.tensor_copy(out=rt0, in_=yss[hh][64:65, :])
                    rt = rp.tile([1, 512], F32, tag="r", name=f"r_{b}{ph}{hh}")
                    nc.vector.reciprocal_approx_fast(rt, rt0)
                    # broadcast r across partitions via a K=1 fp32 ones-matmul
                    # (faster than the DRAM round-trip and off the DMA queues)
                    rb = gpool.tile([128, 512], F32, tag="g",
                                    name=f"rb{b}{ph}{hh}")
                    nc.tensor.matmul(rb, ones_s, rt, start=True, stop=True)
                    # y*r goes straight into its own proj operand tile; the
                    # pe-conv z stays separate and proj accumulates both.
                    nc.vector.tensor_tensor(
                        out=zy[64 * hh:64 * hh + 64, :],
                        in0=yss[hh][0:64, :], in1=rb[0:64, :],
                        op=mybir.AluOpType.mult)
                st["zys"][ph] = zy
                if debug and b == 0:
                    nc.sync.dma_start(dy_d.ap()[ph, 0:65], yss[0])
                    nc.sync.dma_start(dz_d.ap()[ph], st["zts"][ph].bitcast(F32))

            def proj(b):
                st = state[b]
                for mo in range(2):
                    ps = gpool.tile([128, 512], F32, tag="g", name=f"pps{b}_{mo}")
                    for kc in range(2):
                        nc.tensor.matmul(ps, wp_s[:, kc, mo, :], st["zts"][kc],
                                         start=(kc == 0), stop=False)
                    for kc in range(2):
                        nc.tensor.matmul(ps, wp_s[:, kc, mo, :], st["zys"][kc],
                                         start=False, stop=(kc == 1))
                    ot = op_.tile([128, 512], F32, tag="o", name=f"o{b}_{mo}")
                    nc.vector.tensor_scalar_add(out=ot, in0=ps,
                                                scalar1=bp_s[:, mo:mo + 1])
                    nc.sync.dma_start(out_d.ap()[b, mo], ot)

            # cross-batch pipelined emission order: attention for the next
            # unit is emitted before the previous unit's finalize/proj so the
            # PE/ACT never drain at unit boundaries.
            load_weights_crit()
            load_x(0)
            load_weights_bulk()
            qkv(0)
            conv(0, 0)
            conv(0, 1)
            load_x(1)
            qkv(1)
            attn(0, 0)
            norm(0, 0)
            attn(0, 1)
            conv(1, 0)
            conv(1, 1)
            attn(1, 0)
            norm(0, 1)
            proj(0)
            attn(1, 1)
            norm(1, 0)
            norm(1, 1)
            proj(1)

    nc.compile()
    return nc


def _prep_maps(x, qkv_w, qkv_g, qkv_b, qkv_m, qkv_v,
               proj_w, proj_g, proj_b, proj_m, proj_v,
               pe_w, pe_g, pe_b, pe_m, pe_v):
    f = np.float32

    s_qkv = (qkv_g / np.sqrt(qkv_v + EPS)).astype(f)
    t_qkv = (qkv_b - qkv_m * s_qkv).astype(f)
    W = (qkv_w[:, :, 0, 0] * s_qkv[:, None]).astype(f)      # [512, 256]

    hs = np.arange(HEADS)
    qrows = (128 * hs[:, None] + np.arange(KD)[None, :]).ravel()
    krows = qrows + KD
    vrows = (128 * hs[:, None] + 2 * KD + np.arange(HD)[None, :]).ravel()

    scale = f(KD) ** -0.5
    wq_full = (W[qrows] * scale).astype(f)                  # [128, 256]
    bq_full = (t_qkv[qrows] * scale).astype(f)
    wk_full = W[krows]
    bk_full = t_qkv[krows]
    wv_full = W[vrows]                                      # [256, 256], vc=64h+d
    bv_full = t_qkv[vrows]

    def lhst_2(wfull):
        # [O=128, C=256] -> [c, kc, o] with c within 128-chunk kc
        return np.ascontiguousarray(
            wfull.T.reshape(2, 128, 128).transpose(1, 0, 2)).astype(f)

    wq_np = lhst_2(wq_full)
    wk_np = lhst_2(wk_full)

    V = np.zeros((DIM, 260), f)
    bva = np.zeros((1, 260), f)
    for h in range(HEADS):
        V[:, 65 * h:65 * h + 64] = wv_full[64 * h:64 * h + 64].T
        bva[0, 65 * h:65 * h + 64] = bv_full[64 * h:64 * h + 64]
        bva[0, 65 * h + 64] = 1.0
    wva_np = np.ascontiguousarray(
        V.reshape(2, 128, 260).transpose(1, 0, 2)).astype(f)

    def lhst_4(wfull):
        # [O=256, C=256] -> [c, kc, mo, o]
        return np.ascontiguousarray(
            wfull.reshape(2, 128, 2, 128).transpose(3, 2, 0, 1)).astype(f)

    wv_np = lhst_4(wv_full)
    bv_np = np.ascontiguousarray(bv_full.reshape(2, 128).T).astype(f)

    s_pe = (pe_g / np.sqrt(pe_v + EPS)).astype(f)
    t_pe = (pe_b - pe_m * s_pe).astype(f)
    w9_np = np.ascontiguousarray(
        (pe_w[:, 0].reshape(DIM, 9) * s_pe[:, None])
        .reshape(2, 128, 9).transpose(1, 0, 2)).astype(f)

    s_p = (proj_g / np.sqrt(proj_v + EPS)).astype(f)
    t_p = (proj_b - proj_m * s_p).astype(f)
    P_eff = (proj_w[:, :, 0, 0] * s_p[:, None]).astype(f)
    bp_full = (t_p + P_eff @ t_pe).astype(f)
    wp_np = lhst_4(P_eff)
    bp_np = np.ascontiguousarray(bp_full.reshape(2, 128).T).astype(f)

    hmask = np.zeros((128, 4), f)
    for h in range(HEADS):
        hmask[32 * h:32 * h + 32, h] = 1.0

    xf = np.ascontiguousarray(x.reshape(B, DIM, N)).astype(f)
    x_np = np.ascontiguousarray(xf.reshape(B, 2, 128, N))

    shared = dict(
        x=x_np, wk=wk_np, wq=wq_np, wva=wva_np, wv=wv_np, wp=wp_np,
        bk=np.ascontiguousarray(bk_full[:, None]),
        bq=np.ascontiguousarray(bq_full[:, None]),
        bva=bva, bv=bv_np, bp=bp_np, w9=w9_np, hmask=hmask,
    )

    in_maps = []
    for c in range(NCORES):
        own = SLICE * c
        lo, hi = own - 64, own + SLICE + 64
        xr = np.zeros((B, DIM, 640), f)
        a, bnd = max(lo, 0), min(hi, N)
        xr[:, :, a - lo:bnd - lo] = xf[:, :, a:bnd]
        vmask = np.ones((1, 640), f)
        if lo < 0:
            vmask[:, :-lo] = 0.0
        if hi > N:
            vmask[:, 640 - (hi - N):] = 0.0
        m = dict(shared)
        m["xr"] = np.ascontiguousarray(xr.reshape(B, 2, 128, 640))
        m["vmask"] = vmask
        in_maps.append(m)
    return in_maps


def _gather(results):
    full = np.zeros((B, DIM, N), np.float32)
    for c, res in enumerate(results):
        o = res["out"]  # [B, 2, 128, SLICE]
        for mo in range(2):
            full[:, 128 * mo:128 * (mo + 1), SLICE * c:SLICE * (c + 1)] = o[:, mo]
    return full.reshape(B, DIM, 64, 64)


def run(inputs, trace=False, trace_kwargs=None, debug=False):
    from concourse.bass_utils import run_bass_kernel_spmd
    key = ("nc", debug)
    if key not in _CACHE:
        _CACHE[key] = _build_program(debug)
    nc = _CACHE[key]
    in_maps = _prep_maps(**inputs)
    res = run_bass_kernel_spmd(
        nc, in_maps, core_ids=list(range(NCORES)),
        trace=trace, **(trace_kwargs or {}))
    return _gather(res.results), res


def kernel(**inputs):
    inputs = {k: np.asarray(v) for k, v in inputs.items()}
    out, _ = run(inputs, trace=False)
    return out



# revision 5
# speedup vs baseline: 1.1420x; 1.1420x over previous
"""Trainium2 Bass kernel for nn_Attention_19104014533260.

Dense transformer attention block:
  qkv 1x1 conv + BN -> 4-head attention over 4096 pixels (d_k=32, d_v=64)
  -> + depthwise 3x3 conv(v) + BN -> proj 1x1 conv + BN.

Sharding: queries (pixel dim, n=4096) split across 8 cores; each core computes
all heads/batches for its 512-pixel slice. k/v are computed from the full
(replicated) x on every core, so there are no collectives.

v2: fp8 DoubleRow matmuls everywhere on the attention path.
  - S = k8.T @ (q_hi; q_lo) per 128-j chunk via one DoubleRow matmul
    (k read twice through a stride-0 AP dim), 0.5 cycles/row.
  - exp is the wall: split between ScalarE (exact exp -> fp8, scale/bias
    folded) and VectorE (Schraudolph: int8(max(A*S + c, 0)) bitcast as
    e4m3 == 2^((code-56)/8) ~= exp(S - m)). A = 8/ln2 is folded into the
    q weights on the host; softmax shift m cancels in normalization.
  - y = sum_j v8[j,d] e8[j,i] via DoubleRow over j-chunk pairs; a ones
    column appended to v8 makes PSUM row 64 the softmax denominator.
  - qkv path from host-quantized fp8 x (hi only for k/q/va; hi+lo with a
    3-product correction for the pe-conv v, which is error-sensitive).
  - depthwise 3x3 pe conv on GpSimd (9 fused mult-add taps); v bias for
    the attention path folded into the proj bias on the host.
"""

import sys

sys.path.insert(0, "/opt/trn_rl_repo")

import numpy as np

import concourse.bass as bass
import concourse.bacc as bacc_mod
import concourse.tile as tile
from concourse import mybir

F32R = mybir.dt.float32r
F32 = mybir.dt.float32
F8 = mybir.dt.float8e4
I8 = mybir.dt.int8
DR = mybir.MatmulPerfMode.DoubleRow
EPS = 1e-5
NCORES = 8
N = 4096
SLICE = N // NCORES
B = 2
HEADS = 4
KD = 32
HD = 64
DIM = 256

A_SCH = 8.0 / np.log(2.0)        # e4m3 Schraudolph slope, folded into wq
M_SHIFT = 2.5                    # softmax shift; cancels in normalization
C_SCH = 56.0 - A_SCH * M_SHIFT + 0.5

_CACHE = {}


def _dup2(ap2, n=2):
    """Insert a stride-0 dim of size n after the partition dim: [K, M] ->
    [K, n, M] reading the same block n times (DoubleRow k-tile reuse)."""
    dims = [list(p) for p in ap2.ap]
    return bass.AP(tensor=ap2.tensor, offset=ap2.offset,
                   ap=[dims[0], [0, n]] + dims[1:])


def _build_program(debug=False):
    nc = bacc_mod.Bacc()

    xh_d = nc.dram_tensor("xh", [B, 2, 128, N], F8, kind="ExternalInput")
    xrh_d = nc.dram_tensor("xrh", [B, 2, 128, 640], F8, kind="ExternalInput")
    xrl_d = nc.dram_tensor("xrl", [B, 2, 128, 640], F8, kind="ExternalInput")
    wq_d = nc.dram_tensor("wq", [128, 2, 128], F8, kind="ExternalInput")
    wk_d = nc.dram_tensor("wk", [128, 2, 128], F8, kind="ExternalInput")
    wva_d = nc.dram_tensor("wva", [128, 2, 260], F8, kind="ExternalInput")
    wvh_d = nc.dram_tensor("wvh", [128, 2, 2, 128], F8, kind="ExternalInput")
    wvl_d = nc.dram_tensor("wvl", [128, 2, 2, 128], F8, kind="ExternalInput")
    wp_d = nc.dram_tensor("wp", [128, 2, 2, 128], F32R, kind="ExternalInput")
    bqA_d = nc.dram_tensor("bqA", [128, 1], F32, kind="ExternalInput")
    bk_d = nc.dram_tensor("bk", [128, 1], F32, kind="ExternalInput")
    bv_d = nc.dram_tensor("bv", [128, 2], F32, kind="ExternalInput")
    bp_d = nc.dram_tensor("bp", [128, 2], F32, kind="ExternalInput")
    w9_d = nc.dram_tensor("w9", [128, 2, 9], F32, kind="ExternalInput")
    vmask_d = nc.dram_tensor("vmask", [1, 640], F32, kind="ExternalInput")
    out_d = nc.dram_tensor("out", [B, 2, 128, SLICE], F32, kind="ExternalOutput")
    if debug:
        dk_d = nc.dram_tensor("dk", [128, 8, 512], F8, kind="ExternalOutput")
        dq_d = nc.dram_tensor("dq", [128, 2, 512], F8, kind="ExternalOutput")
        dva_d = nc.dram_tensor("dva", [128, 16, 2, 260], F8, kind="ExternalOutput")
        de_d = nc.dram_tensor("de", [128, 2, 2, 1024], F8, kind="ExternalOutput")
        dy_d = nc.dram_tensor("dy", [2, 65, 512], F32, kind="ExternalOutput")
        dz_d = nc.dram_tensor("dz", [2, 128, 512], F32, kind="ExternalOutput")
        dzy_d = nc.dram_tensor("dzy", [2, 128, 512], F32, kind="ExternalOutput")

    def bcast_rows(dram_ap, rows):
        return bass.AP(
            tensor=dram_ap.tensor,
            offset=dram_ap.offset,
            ap=[[0, rows]] + [list(p) for p in dram_ap.ap[1:]],
        )

    # exp engine schedule per jc chunk (32 per unit): 'A' ScalarE, 'D' VectorE
    EXP_PAT = "ADADADAD" * 4

    with tile.TileContext(nc) as tc:
        with (
            tc.tile_pool(name="singles", bufs=1) as singles,
            tc.tile_pool(name="xp", bufs=8) as xp,
            tc.tile_pool(name="xrp", bufs=2) as xrp,
            tc.tile_pool(name="kp", bufs=16) as kp,
            tc.tile_pool(name="qp", bufs=2) as qp,
            tc.tile_pool(name="ep", bufs=6) as ep,
            tc.tile_pool(name="vpp", bufs=3) as vpp,
            tc.tile_pool(name="zp", bufs=8) as zp,
            tc.tile_pool(name="rp", bufs=4) as rp,
            tc.tile_pool(name="op", bufs=4) as op_,
            tc.tile_pool(name="spool", bufs=2, space="PSUM") as spool,
            tc.tile_pool(name="ypool", bufs=2, space="PSUM") as ypool,
            tc.tile_pool(name="gpool", bufs=2, space="PSUM") as gpool,
        ):
            wq_s = singles.tile([128, 2, 128], F8, tag="wq")
            wk_s = singles.tile([128, 2, 128], F8, tag="wk")
            wva_s = singles.tile([128, 2, 260], F8, tag="wva")
            wvh_s = singles.tile([128, 2, 2, 128], F8, tag="wvh")
            wvl_s = singles.tile([128, 2, 2, 128], F8, tag="wvl")
            wp_s = singles.tile([128, 2, 2, 128], F32R, tag="wp")
            bqA_s = singles.tile([128, 1], F32, tag="bqA")
            bk_s = singles.tile([128, 1], F32, tag="bk")
            bv_s = singles.tile([128, 2], F32, tag="bv")
            bp_s = singles.tile([128, 2], F32, tag="bp")
            w9_s = singles.tile([128, 2, 9], F32, tag="w9")
            vmask_s = singles.tile([128, 640], F32, tag="vmask")
            ones_s = singles.tile([1, 128], F32R, tag="ones")
            mbias_s = singles.tile([128, 1], F32, tag="mbias")
            # persistent v^T store: [b, jc-pair, jc%2, 4*(64+1)]; ones cols
            # at 65h+64 are written once and casts skip them.
            va8_s = singles.tile([128, B, 16, 2, 260], F8, tag="va8")

            nc.vector.memset(ones_s, 1.0)
            nc.vector.memset(mbias_s, -M_SHIFT)
            for b in range(B):
                nc.gpsimd.memset(
                    va8_s[:, b].rearrange("p a t (h c) -> p a t h c", c=65)
                    [:, :, :, :, 64:65], 1.0)

            def load_weights_crit():
                for t, d in [(wq_s, wq_d), (wk_s, wk_d), (bqA_s, bqA_d),
                             (bk_s, bk_d)]:
                    nc.sync.dma_start(t, d.ap())

            def load_weights_bulk():
                for t, d in [(wva_s, wva_d), (wvh_s, wvh_d), (wvl_s, wvl_d),
                             (wp_s, wp_d), (bv_s, bv_d), (bp_s, bp_d),
                             (w9_s, w9_d)]:
                    nc.gpsimd.dma_start(t, d.ap())
                nc.gpsimd.dma_start(vmask_s, bcast_rows(vmask_d.ap(), 128))

            TAPS = [(dy, dx) for dy in (-1, 0, 1) for dx in (-1, 0, 1)]
            state = {}

            def load_x(b):
                xrht = xrp.tile([128, 2, 640], F8, tag="xrh", name=f"xrh{b}")
                xrlt = xrp.tile([128, 2, 640], F8, tag="xrl", name=f"xrl{b}")
                nc.gpsimd.dma_start(
                    xrht, xrh_d.ap()[b].rearrange("a p c -> p a c"))
                nc.gpsimd.dma_start(
                    xrlt, xrl_d.ap()[b].rearrange("a p c -> p a c"))
                xts = []
                for n8 in range(8):
                    xt = xp.tile([128, 2, 512], F8, tag="x", name=f"x{b}_{n8}")
                    nc.sync.dma_start(
                        xt, xh_d.ap()[b, :, :, n8 * 512:(n8 + 1) * 512]
                        .rearrange("a p c -> p a c"))
                    xts.append(xt)
                state[b] = dict(xrht=xrht, xrlt=xrlt, xts=xts)

            def qk(b):
                st = state[b]
                # q: one DoubleRow matmul (kc tiles), then hi/lo fp8 split
                psq = gpool.tile([128, 512], F32, tag="g", name=f"qps{b}")
                nc.tensor.matmul(psq, wq_s[:, :, :],
                                 st["xrht"][:, :, 64:576],
                                 start=True, stop=True, perf_mode=DR)
                q8t = qp.tile([128, 2, 512], F8, tag="q", name=f"q{b}")
                nc.scalar.activation(q8t[:, 0, :], psq,
                                     mybir.ActivationFunctionType.Identity,
                                     bias=bqA_s[:, :])
                nc.vector.scalar_tensor_tensor(
                    out=q8t[:, 1, :], in0=psq, scalar=bqA_s[:, :],
                    in1=q8t[:, 0, :],
                    op0=mybir.AluOpType.add, op1=mybir.AluOpType.subtract)
                st["q8"] = q8t
                kts = []
                for n8 in range(8):
                    ps = gpool.tile([128, 512], F32, tag="g", name=f"kps{b}_{n8}")
                    nc.tensor.matmul(ps, wk_s[:, :, :], st["xts"][n8],
                                     start=True, stop=True, perf_mode=DR)
                    ktc = kp.tile([128, 512], F8, tag="k", name=f"k{b}_{n8}")
                    nc.scalar.activation(ktc, ps,
                                         mybir.ActivationFunctionType.Identity,
                                         bias=bk_s[:, :])
                    kts.append(ktc)
                st["kts"] = kts

            def va_chunk(b, n32):
                st = state[b]
                n8, sub = divmod(n32, 4)
                ps = gpool.tile([128, 512], F32, tag="g", name=f"vaps{b}_{n32}")
                nc.tensor.matmul(
                    ps[:, 0:260],
                    st["xts"][n8][:, :, sub * 128:(sub + 1) * 128],
                    wva_s[:, :, :], start=True, stop=True, perf_mode=DR)
                src = ps[:, 0:260].rearrange("p (h c) -> p h c", c=65)[:, :, 0:64]
                dst = (va8_s[:, b, n32 // 2, n32 % 2]
                       .rearrange("p (h c) -> p h c", c=65)[:, :, 0:64])
                if n32 % 2 == 0:
                    nc.scalar.activation(
                        dst, src, mybir.ActivationFunctionType.Copy)
                else:
                    nc.vector.tensor_copy(out=dst, in_=src)

            def vpe(b):
                st = state[b]
                vps = []
                for mc in range(2):
                    vpt = vpp.tile([128, 660], F32R, tag="vp", name=f"vp{b}_{mc}")
                    nc.vector.memset(vpt, 0.0)
                    vpv = vpt.rearrange("p (r c) -> p r c", c=66)
                    for (c0, w, r0, nr) in [(0, 512, 0, 8), (512, 128, 8, 2)]:
                        ps = gpool.tile([128, 512], F32, tag="g",
                                        name=f"vps{b}_{mc}_{c0}")
                        for i, (wv, xr) in enumerate([
                                (wvh_s, st["xrht"]), (wvl_s, st["xrht"]),
                                (wvh_s, st["xrlt"])]):
                            nc.tensor.matmul(
                                ps[:, 0:w], wv[:, :, mc, :],
                                xr[:, :, c0:c0 + w],
                                start=(i == 0), stop=(i == 2), perf_mode=DR)
                        nc.vector.scalar_tensor_tensor(
                            out=vpv[:, r0:r0 + nr, 1:65],
                            in0=ps[:, 0:w].rearrange("p (r c) -> p r c", c=64),
                            scalar=bv_s[:, mc:mc + 1],
                            in1=vmask_s[:, c0:c0 + w]
                            .rearrange("p (r c) -> p r c", c=64),
                            op0=mybir.AluOpType.add, op1=mybir.AluOpType.mult)
                    vps.append(vpt)
                st["vps"] = vps

            def conv(b, ph):
                st = state[b]
                zt = zp.tile([128, 512], F32R, tag="z", name=f"z{b}_{ph}")
                zv = zt.rearrange("p (r c) -> p r c", c=64)
                vpv = st["vps"][ph].rearrange("p (r c) -> p r c", c=66)
                for t, (dy, dx) in enumerate(TAPS):
                    if t == 0:
                        nc.gpsimd.tensor_scalar(
                            out=zv, in0=vpv[:, 1 + dy:9 + dy, 1 + dx:65 + dx],
                            scalar1=w9_s[:, ph, t:t + 1], scalar2=None,
                            op0=mybir.AluOpType.mult)
                    else:
                        nc.gpsimd.scalar_tensor_tensor(
                            out=zv,
                            in0=vpv[:, 1 + dy:9 + dy, 1 + dx:65 + dx],
                            scalar=w9_s[:, ph, t:t + 1], in1=zv,
                            op0=mybir.AluOpType.mult,
                            op1=mybir.AluOpType.add)
                st.setdefault("zts", {})[ph] = zt

            def attn(b, ph, weave=None):
                st = state[b]
                kts = st["kts"]
                q8t = st["q8"]
                yts = [ypool.tile([128, 512], F32, tag="y",
                                  name=f"y{b}_{ph}_{hh}") for hh in range(2)]
                epair = None
                for jc in range(32):
                    j8, jsub = divmod(jc, 4)
                    sp = spool.tile([128, 1024], F32, tag="s",
                                    name=f"s{b}_{ph}_{jc}")
                    for hh in range(2):
                        h = 2 * ph + hh
                        lhsT = _dup2(
                            kts[j8][32 * h:32 * h + 32,
                                    jsub * 128:(jsub + 1) * 128])
                        nc.tensor.matmul(
                            sp[:, hh * 512:(hh + 1) * 512], lhsT,
                            q8t[32 * h:32 * h + 32, :, :],
                            start=True, stop=True, perf_mode=DR,
                            tile_position=(32 * h, 0))
                    if jc % 2 == 0:
                        epair = ep.tile([128, 2, 1024], F8, tag="e",
                                        name=f"e{b}_{ph}_{jc // 2}")
                    dst = epair[:, jc % 2, :]
                    if EXP_PAT[jc] == "A":
                        nc.scalar.activation(
                            dst, sp, mybir.ActivationFunctionType.Exp,
                            bias=mbias_s[:, :], scale=float(1.0 / A_SCH))
                    else:
                        nc.vector.tensor_scalar(
                            out=epair.bitcast(I8)[:, jc % 2, :], in0=sp,
                            scalar1=float(C_SCH), scalar2=0.0,
                            op0=mybir.AluOpType.add, op1=mybir.AluOpType.max)
                    if jc % 2 == 1:
                        p = jc // 2
                        for hh in range(2):
                            h = 2 * ph + hh
                            nc.tensor.matmul(
                                yts[hh][0:65, :],
                                va8_s[:, b, p, :, 65 * h:65 * h + 65],
                                epair[:, :, hh * 512:(hh + 1) * 512],
                                start=(p == 0), stop=(p == 15), perf_mode=DR)
                        if debug and b == 0:
                            nc.sync.dma_start(de_d.ap()[:, ph, :], epair)
                    if weave is not None:
                        weave(jc)
                st[f"y{ph}"] = yts

            def norm(b, ph):
                st = state[b]
                yts = st[f"y{ph}"]
                zy = zp.tile([128, 512], F32R, tag="z", name=f"zy{b}_{ph}")
                for hh in range(2):
                    rt0 = rp.tile([1, 512], F32, tag="r0", name=f"r0_{b}{ph}{hh}")
                    nc.vector.tensor_copy(out=rt0, in_=yts[hh][64:65, :])
                    rt = rp.tile([1, 512], F32, tag="r", name=f"r_{b}{ph}{hh}")
                    nc.vector.reciprocal_approx_fast(rt, rt0)
                    rb = gpool.tile([128, 512], F32, tag="g",
                                    name=f"rb{b}{ph}{hh}")
                    nc.tensor.matmul(rb, ones_s, rt.bitcast(F32R),
                                     start=True, stop=True)
                    nc.vector.tensor_tensor(
                        out=zy[64 * hh:64 * hh + 64, :],
                        in0=yts[hh][0:64, :], in1=rb[0:64, :],
                        op=mybir.AluOpType.mult)
                st.setdefault("zys", {})[ph] = zy
                if debug and b == 0 and ph == 0:
                    nc.sync.dma_start(dy_d.ap()[0, :, :], yts[0][0:65, :])

            def proj(b):
                st = state[b]
                for mo in range(2):
                    ps = gpool.tile([128, 512], F32, tag="g", name=f"pps{b}_{mo}")
                    for kc in range(2):
                        nc.tensor.matmul(ps, wp_s[:, kc, mo, :], st["zts"][kc],
                                         start=(kc == 0), stop=False)
                    for kc in range(2):
                        nc.tensor.matmul(ps, wp_s[:, kc, mo, :], st["zys"][kc],
                                         start=False, stop=(kc == 1))
                    ot = op_.tile([128, 512], F32, tag="o", name=f"o{b}_{mo}")
                    nc.scalar.activation(
                        ot, ps, mybir.ActivationFunctionType.Identity,
                        bias=bp_s[:, mo:mo + 1])
                    nc.sync.dma_start(out_d.ap()[b, mo], ot)
                if debug and b == 0:
                    nc.sync.dma_start(dz_d.ap()[0], st["zts"][0].bitcast(F32))
                    nc.sync.dma_start(dzy_d.ap()[0], st["zys"][0].bitcast(F32))

            # ---- emission schedule (cross-unit pipelined) ----
            load_weights_crit()
            load_x(0)
            load_weights_bulk()
            qk(0)
            vpe(0)
            conv(0, 0)
            conv(0, 1)

            # weave va(0) into attn(0,0): va pair p is consumed by the
            # y-matmul right after e-pair p completes; emit chunks 4 ahead.
            for n32 in range(4):
                va_chunk(0, n32)
            attn(0, 0, weave=lambda jc: (
                va_chunk(0, jc + 4) if jc < 28 else None))
            load_x(1)
            if debug:
                for n8 in range(8):
                    nc.sync.dma_start(dk_d.ap()[:, n8], state[0]["kts"][n8])
                nc.sync.dma_start(dq_d.ap(), state[0]["q8"])
                for p16 in range(16):
                    nc.sync.dma_start(dva_d.ap()[:, p16], va8_s[:, 0, p16])
            qk(1)
            attn(0, 1, weave=lambda jc: va_chunk(1, jc))
            norm(0, 0)
            vpe(1)
            attn(1, 0)
            conv(1, 0)
            conv(1, 1)
            norm(0, 1)
            proj(0)
            attn(1, 1)
            norm(1, 0)
            norm(1, 1)
            proj(1)

    nc.compile()
    return nc


def _prep_maps(x, qkv_w, qkv_g, qkv_b, qkv_m, qkv_v,
               proj_w, proj_g, proj_b, proj_m, proj_v,
               pe_w, pe_g, pe_b, pe_m, pe_v):
    import ml_dtypes
    E4 = ml_dtypes.float8_e4m3
    f = np.float32

    def q8np(a):
        return np.clip(a, -240, 240).astype(E4)

    def split8(a):
        h = q8np(a)
        l = q8np(a - h.astype(f))
        return h, l

    s_qkv = (qkv_g / np.sqrt(qkv_v + EPS)).astype(f)
    t_qkv = (qkv_b - qkv_m * s_qkv).astype(f)
    W = (qkv_w[:, :, 0, 0] * s_qkv[:, None]).astype(f)      # [512, 256]

    hs = np.arange(HEADS)
    qrows = (128 * hs[:, None] + np.arange(KD)[None, :]).ravel()
    krows = qrows + KD
    vrows = (128 * hs[:, None] + 2 * KD + np.arange(HD)[None, :]).ravel()

    scale = f(KD) ** -0.5
    qmul = f(scale * A_SCH)
    wq_full = (W[qrows] * qmul).astype(f)                   # [128, 256]
    bqA_full = (t_qkv[qrows] * qmul).astype(f)
    wk_full = W[krows]
    bk_full = t_qkv[krows]
    wv_full = W[vrows]                                      # [256, 256]
    bv_full = t_qkv[vrows]

    def lhst_2(wfull):
        # [O=128, C=256] -> [c, kc, o]
        return np.ascontiguousarray(
            wfull.T.reshape(2, 128, 128).transpose(1, 0, 2)).astype(f)

    wq_np = q8np(lhst_2(wq_full))
    wk_np = q8np(lhst_2(wk_full))

    V = np.zeros((DIM, 260), f)
    for h in range(HEADS):
        V[:, 65 * h:65 * h + 64] = wv_full[64 * h:64 * h + 64].T
    wva_np = q8np(np.ascontiguousarray(
        V.reshape(2, 128, 260).transpose(1, 0, 2)))

    def lhst_4(wfull):
        # [O=256, C=256] -> [c, kc, mo, o]
        return np.ascontiguousarray(
            wfull.reshape(2, 128, 2, 128).transpose(3, 2, 0, 1)).astype(f)

    wv_lhst = lhst_4(wv_full)
    wvh_np, wvl_np = split8(wv_lhst)
    bv_np = np.ascontiguousarray(bv_full.reshape(2, 128).T).astype(f)

    s_pe = (pe_g / np.sqrt(pe_v + EPS)).astype(f)
    t_pe = (pe_b - pe_m * s_pe).astype(f)
    w9_np = np.ascontiguousarray(
        (pe_w[:, 0].reshape(DIM, 9) * s_pe[:, None])
        .reshape(2, 128, 9).transpose(1, 0, 2)).astype(f)

    s_p = (proj_g / np.sqrt(proj_v + EPS)).astype(f)
    t_p = (proj_b - proj_m * s_p).astype(f)
    P_eff = (proj_w[:, :, 0, 0] * s_p[:, None]).astype(f)
    # v bias for the attention path rides the proj bias (y + bv before proj)
    bp_full = (t_p + P_eff @ t_pe + P_eff @ bv_full).astype(f)
    wp_np = lhst_4(P_eff)
    bp_np = np.ascontiguousarray(bp_full.reshape(2, 128).T).astype(f)

    xf = np.ascontiguousarray(x.reshape(B, DIM, N)).astype(f)
    xh8 = q8np(xf)
    xh_np = np.ascontiguousarray(xh8.reshape(B, 2, 128, N))

    shared = dict(
        xh=xh_np, wq=wq_np, wk=wk_np, wva=wva_np, wvh=wvh_np, wvl=wvl_np,
        wp=wp_np,
        bqA=np.ascontiguousarray(bqA_full[:, None]),
        bk=np.ascontiguousarray(bk_full[:, None]),
        bv=bv_np, bp=bp_np, w9=w9_np,
    )

    in_maps = []
    for c in range(NCORES):
        own = SLICE * c
        lo, hi = own - 64, own + SLICE + 64
        xr = np.zeros((B, DIM, 640), f)
        a, bnd = max(lo, 0), min(hi, N)
        xr[:, :, a - lo:bnd - lo] = xf[:, :, a:bnd]
        xrh8, xrl8 = split8(xr)
        vmask = np.ones((1, 640), f)
        if lo < 0:
            vmask[:, :-lo] = 0.0
        if hi > N:
            vmask[:, 640 - (hi - N):] = 0.0
        m = dict(shared)
        m["xrh"] = np.ascontiguousarray(xrh8.reshape(B, 2, 128, 640))
        m["xrl"] = np.ascontiguousarray(xrl8.reshape(B, 2, 128, 640))
        m["vmask"] = vmask
        in_maps.append(m)
    return in_maps


def _gather(results):
    full = np.zeros((B, DIM, N), np.float32)
    for c, res in enumerate(results):
        o = res["out"]  # [B, 2, 128, SLICE]
        for mo in range(2):
            full[:, 128 * mo:128 * (mo + 1), SLICE * c:SLICE * (c + 1)] = o[:, mo]
    return full.reshape(B, DIM, 64, 64)


def run(inputs, trace=False, trace_kwargs=None, debug=False):
    from concourse.bass_utils import run_bass_kernel_spmd
    key = ("nc", debug)
    if key not in _CACHE:
        _CACHE[key] = _build_program(debug)
    nc = _CACHE[key]
    in_maps = _prep_maps(**inputs)
    res = run_bass_kernel_spmd(
        nc, in_maps, core_ids=list(range(NCORES)),
        trace=trace, **(trace_kwargs or {}))
    return _gather(res.results), res


def kernel(**inputs):
    inputs = {k: np.asarray(v) for k, v in inputs.items()}
    out, _ = run(inputs, trace=False)
    return out
